# revision 9
# baseline (speedup 1.0000x reference)
"""Trainium2 Bass kernel for nn_Causal_Temporal_Map_Attention_2.

Reference computation (B=16, N=2048, T=512, E=64):
    W0m  = relu(triu(W0, 1))
    attn = (x@W0m.T)@x.T + (x@W1.T)@e.T + (e@W2.T)@x.T + (e@W3.T)@e.T
    out  = attn @ x

Associativity avoids the [B, N, N] attention map:
    G[b] = x[b].T @ x[b]                  # [512, 512]  (symmetric!)
    H[b] = e.T @ x[b]                     # [64, 512]
    M0[b] = W0m.T @ G[b] + W1.T @ H[b]    # [512, 512]
    M1[b] = W2.T @ G[b] + W3.T @ H[b]     # [64, 512]
    out[b] = x[b] @ M0[b] + e @ M1[b]     # [2048, 512]

Stage B exploits G's symmetry: only the upper-triangular blocks are
computed, and [x | e] is stored adjacently so the H^T columns ride
along in the same matmuls; the 5 mirror blocks are filled by cheap PE
transposes.  Rows streamed per k-chunk: 1664 vs 2560 naive.

Everything runs in bf16 on the PE (fp32 PSUM accumulation): same
streaming rate as f32r, no small-free-dim penalty, 2x faster weight
loads (FWL), half the SBUF traffic.  Max rel err ~3e-3.

Sharding: data-parallel over batch, 2 batches per core on 8 cores.
"""

import numpy as np

import concourse.bass as bass
import concourse.mybir as mybir
import concourse.tile as tile
from concourse import bacc
from concourse.bass import ts
from concourse.bass_utils import run_bass_kernel_spmd
from concourse.masks import make_identity

N_CORES = 8
B = 16
B2 = B // N_CORES  # batches per core
N = 2048
T = 512
E = 64
NCHUNKS = N // 128  # 16
KT = T // 128  # 4

f32 = mybir.dt.float32
bf16 = mybir.dt.bfloat16
AF = mybir.ActivationFunctionType


def build_module():
    nc = bacc.Bacc("TRN2", target_bir_lowering=False, debug=False, num_devices=N_CORES)

    X = nc.dram_tensor("x", [B2, N, T], f32, kind="ExternalInput").ap()
    Ein = nc.dram_tensor("e", [N, E], f32, kind="ExternalInput").ap()
    W0 = nc.dram_tensor("W0", [T, T], f32, kind="ExternalInput").ap()
    W1 = nc.dram_tensor("W1", [E, T], f32, kind="ExternalInput").ap()
    W2 = nc.dram_tensor("W2", [T, E], f32, kind="ExternalInput").ap()
    W3 = nc.dram_tensor("W3", [E, E], f32, kind="ExternalInput").ap()
    OUT = nc.dram_tensor("out", [B2, N, T], f32, kind="ExternalOutput").ap()

    TE = T + E  # 576: x chunk with e columns appended

    with tile.TileContext(nc) as tc:
        with (
            tc.tile_pool(name="const", bufs=1) as cpool,
            tc.tile_pool(name="xst", bufs=6) as xstpool,
            tc.tile_pool(name="gh", bufs=1) as ghpool,
            tc.tile_pool(name="m01", bufs=1) as mpool,
            tc.tile_pool(name="xt", bufs=4) as xtpool,
            tc.tile_pool(name="outst", bufs=4) as opool,
            # 5 single-bank accumulators shared by stages B/C/D + 3 banks
            # for PE-transpose results: 8 banks total.
            tc.tile_pool(name="psacc", bufs=1, space="PSUM") as psacc,
            tc.tile_pool(name="pst", bufs=3, space="PSUM") as pst,
        ):
            def _scalar_copy(out, in_):
                nc.scalar.activation(out, in_, AF.Copy)

            def _vector_copy(out, in_):
                nc.vector.tensor_copy(out, in_)

            def _gpsimd_copy(out, in_):
                nc.gpsimd.tensor_copy(out, in_)

            copy_engines = [_vector_copy, _gpsimd_copy, _scalar_copy]
            copy_ps = [_vector_copy, _scalar_copy]  # PSUM-capable engines

            # ---- persistent [x | e] staging in bf16: one set per batch
            xe = [
                [
                    cpool.tile([128, TE], bf16, tag=f"xe{s}_{k}", name=f"xe{s}_{k}")
                    for k in range(NCHUNKS)
                ]
                for s in range(B2)
            ]

            # e first (small): one strided DMA + converts into batch-0 set
            e_st = cpool.tile([128, NCHUNKS, E], f32)
            nc.sync.dma_start(e_st[:], Ein.rearrange("(a p) k -> p a k", p=128))
            for k in range(NCHUNKS):
                _gpsimd_copy(xe[0][k][:, T:TE], e_st[:, k, :])
            for k in range(NCHUNKS):
                _gpsimd_copy(xe[1][k][:, T:TE], e_st[:, k, :])

            # batch-0 x chunks: DMA to f32 staging, convert to bf16
            def load_x(s):
                for k in range(NCHUNKS):
                    xs = xstpool.tile([128, T], f32, tag="xst", name="xs")
                    nc.sync.dma_start(xs[:], X[s, ts(k, 128), :])
                    _gpsimd_copy(xe[s][k][:, 0:T], xs[:])

            load_x(0)

            ident32 = cpool.tile([128, 128], f32)
            make_identity(nc, ident32[:])
            identb = cpool.tile([128, 128], bf16)
            nc.vector.tensor_copy(identb[:], ident32[:])

            # ---- weights (needed by stage C, ~25us in) ----
            w0_st = cpool.tile([128, KT, T], f32)
            for kt in range(KT):
                nc.sync.dma_start(w0_st[:, kt, :], W0[ts(kt, 128), :])
            w0m = cpool.tile([128, KT, T], bf16)
            for kt in range(KT):
                # keep W0[d, t] iff t >= d+1  (d = p + 128*kt)
                nc.gpsimd.affine_select(
                    out=w0_st[:, kt, :],
                    in_=w0_st[:, kt, :],
                    compare_op=mybir.AluOpType.is_ge,
                    fill=0.0,
                    base=-(128 * kt + 1),
                    pattern=[[1, T]],
                    channel_multiplier=-1,
                )
                nc.scalar.activation(w0m[:, kt, :], w0_st[:, kt, :], AF.Relu)

            w1_st = cpool.tile([E, T], f32)
            nc.sync.dma_start(w1_st[:], W1[:])
            w1 = cpool.tile([E, T], bf16)
            nc.vector.tensor_copy(w1[:], w1_st[:])
            w2_st = cpool.tile([128, KT, E], f32)
            for kt in range(KT):
                nc.sync.dma_start(w2_st[:, kt, :], W2[ts(kt, 128), :])
            w2 = cpool.tile([128, KT, E], bf16)
            nc.gpsimd.tensor_copy(w2[:], w2_st[:])
            w3_st = cpool.tile([E, E], f32)
            nc.sync.dma_start(w3_st[:], W3[:])
            w3 = cpool.tile([E, E], bf16)
            nc.vector.tensor_copy(w3[:], w3_st[:])

            # batch-1 x after the weights on the DMA queue
            load_x(1)

            def xr(s, k, c0, c1):
                return xe[s][k][:, c0:c1]

            # ---- e^T in bf16 for stage D's e-term (one-time) ----
            etr = cpool.tile([E, NCHUNKS, 128], bf16)
            for g in range(4):
                pte = pst.tile([128, 512], bf16, tag="pt", name="pte")
                for j in range(4):
                    k = g * 4 + j
                    nc.tensor.transpose(pte[:E, ts(j, 128)], xr(0, k, T, TE), identb[:])
                nc.vector.tensor_copy(
                    etr[:, ts(g, 4), :].rearrange("p a q -> p (a q)"), pte[:E, :]
                )

            # ---------------- per batch ----------------
            for b in range(B2):
                # Stage B: symmetric G = x^T x upper blocks + H^T columns.
                # Per k-chunk, 5 matmuls (lhsT = x block i, rhs spans
                # [x cols | e cols] so H^T rides along):
                #   mA: i=0, cols [0,256)        -> G(0,0),G(0,1)
                #   mB: i=0, cols [256,576)      -> G(0,2),G(0,3),HT0
                #   mC: i=1, cols [128,576)      -> G(1,1..3),HT1
                #   mD: i=2, cols [256,576)      -> G(2,2),G(2,3),HT2
                #   mE: i=3, cols [256,576)      -> G(3,2),G(3,3),HT3
                pa = psacc.tile([128, 256], f32, tag="pA", name="pa")
                pb = psacc.tile([128, 320], f32, tag="pB", name="pb")
                pc = psacc.tile([128, 448], f32, tag="pC", name="pc")
                pd = psacc.tile([128, 320], f32, tag="pD", name="pd")
                pe = psacc.tile([128, 320], f32, tag="pE", name="pe")
                for k in range(NCHUNKS):
                    st = dict(start=(k == 0), stop=(k == NCHUNKS - 1))
                    nc.tensor.matmul(pa[:], xr(b, k, 0, 128), xr(b, k, 0, 256), **st)
                    nc.tensor.matmul(pb[:], xr(b, k, 0, 128), xr(b, k, 256, TE), **st)
                    nc.tensor.matmul(pc[:], xr(b, k, 128, 256), xr(b, k, 128, TE), **st)
                    nc.tensor.matmul(pd[:], xr(b, k, 256, 384), xr(b, k, 256, TE), **st)
                    nc.tensor.matmul(pe[:], xr(b, k, 384, 512), xr(b, k, 256, TE), **st)

                gh = [
                    ghpool.tile([128, T], bf16, tag=f"g{i}", name=f"g{i}")
                    for i in range(KT)
                ]
                hh = ghpool.tile([E, T], bf16, tag="h", name="h")
                # direct blocks (psum f32 -> sbuf bf16 cast copies)
                nc.vector.tensor_copy(gh[0][:, 0:256], pa[:])
                _scalar_copy(gh[0][:, 256:512], pb[:, 0:256])
                nc.vector.tensor_copy(gh[1][:, 128:512], pc[:, 0:384])
                _scalar_copy(gh[2][:, 256:512], pd[:, 0:256])
                nc.vector.tensor_copy(gh[3][:, 256:512], pe[:, 0:256])
                # mirror blocks via PE transpose (SBUF -> psum -> copy)
                mirrors = [
                    (0, 128, 1, 0),  # G(0,1)^T -> gh1 cols 0:128
                    (0, 256, 2, 0),  # G(0,2)^T -> gh2 cols 0:128
                    (1, 256, 2, 128),  # G(1,2)^T -> gh2 cols 128:256
                    (0, 384, 3, 0),  # G(0,3)^T -> gh3 cols 0:128
                    (1, 384, 3, 128),  # G(1,3)^T -> gh3 cols 128:256
                ]
                ptm = pst.tile([128, 512], bf16, tag="pt", name="ptm")
                for mi, (src, scol, dst, dcol) in enumerate(mirrors[:4]):
                    nc.tensor.transpose(
                        ptm[:, ts(mi, 128)], gh[src][:, scol : scol + 128], identb[:]
                    )
                for mj, (s2, sc2, d2, dc2) in enumerate(mirrors[:4]):
                    copy_ps[mj % 2](
                        gh[d2][:, dc2 : dc2 + 128], ptm[:, ts(mj, 128)]
                    )
                ptm2 = pst.tile([128, 512], bf16, tag="pt", name="ptm2")
                src, scol, dst, dcol = mirrors[4]
                nc.tensor.transpose(
                    ptm2[:, 0:128], gh[src][:, scol : scol + 128], identb[:]
                )
                _vector_copy(gh[3][:, 128:256], ptm2[:, 0:128])
                # H [64, 512] from the HT columns (transpose each block).
                # HT blocks live in PSUM; transpose input must be SBUF, so
                # stage the 64-wide tails through a scratch bf16 tile.
                htmp = ghpool.tile([128, KT, E], bf16, tag="htmp", name="htmp")
                hsrc = [(pb, 256), (pc, 384), (pd, 256), (pe, 256)]
                for i, (src_t, c0) in enumerate(hsrc):
                    copy_ps[i % 2](htmp[:, i, :], src_t[:, c0 : c0 + E])
                pth = pst.tile([128, 512], bf16, tag="pt", name="pth")
                for i in range(KT):
                    nc.tensor.transpose(pth[:E, ts(i, 128)], htmp[:, i, :], identb[:])
                nc.scalar.activation(hh[:], pth[:E, :], AF.Copy)

                # Stage C: M0 = W0m^T G + W1^T H ; M1 = W2^T G + W3^T H
                m0 = [
                    mpool.tile([128, T], bf16, tag=f"m0{mc}", name=f"m0{mc}")
                    for mc in range(KT)
                ]
                m1 = mpool.tile([E, T], bf16, tag="m1", name="m1")
                ctags = ["pA", "pB", "pD", "pE"]
                for mc in range(KT):
                    pm = psacc.tile([128, T], f32, tag=ctags[mc], name=f"pm{mc}")
                    # W0m strictly upper triangular: block (kt, mc) is
                    # zero unless kt <= mc.
                    for kt in range(mc + 1):
                        nc.tensor.matmul(
                            pm[:],
                            w0m[:, kt, ts(mc, 128)],
                            gh[kt][:],
                            start=(kt == 0),
                            stop=False,
                        )
                    nc.tensor.matmul(
                        pm[:], w1[:, ts(mc, 128)], hh[:], start=False, stop=True
                    )
                    copy_ps[mc % 2](m0[mc][:], pm[:])
                pm1 = psacc.tile([128, T], f32, tag="pC", name="pm1")
                for kt in range(KT):
                    nc.tensor.matmul(
                        pm1[:E, :], w2[:, kt, :], gh[kt][:], start=(kt == 0), stop=False
                    )
                nc.tensor.matmul(pm1[:E, :], w3[:], hh[:], start=False, stop=True)
                nc.scalar.activation(m1[:], pm1[:E, :], AF.Copy)

                # Stage D: out = x @ M0 + e @ M1, chunk by chunk over n.
                dtags = ["pA", "pB", "pC", "pD", "pE"]
                for i in range(NCHUNKS):
                    ptx = pst.tile([128, 512], bf16, tag="pt", name="ptx")
                    for kt in range(KT):
                        nc.tensor.transpose(
                            ptx[:, ts(kt, 128)],
                            xr(b, i, kt * 128, (kt + 1) * 128),
                            identb[:],
                        )
                    xt = xtpool.tile([128, KT, 128], bf16, tag="xt", name="xt")
                    copy_ps[i % 2](
                        xt[:].rearrange("p a q -> p (a q)"), ptx[:]
                    )
                    po = psacc.tile([128, T], f32, tag=dtags[i % 5], name="po")
                    nc.tensor.matmul(po[:], etr[:, i, :], m1[:], start=True, stop=False)
                    for kt in range(KT):
                        nc.tensor.matmul(
                            po[:],
                            xt[:, kt, :],
                            m0[kt][:],
                            start=False,
                            stop=(kt == KT - 1),
                        )
                    ot = opool.tile([128, T], f32, tag="ot", name="ot")
                    copy_ps[(i + 1) % 2](ot[:], po[:])
                    nc.sync.dma_start(OUT[b, ts(i, 128), :], ot[:])

    nc.compile()
    return nc


_CACHE = {}


def _get_module():
    if "nc" not in _CACHE:
        _CACHE["nc"] = build_module()
    return _CACHE["nc"]


def _run(nc, in_maps, tries=3):
    last = None
    for _ in range(tries):
        try:
            return run_bass_kernel_spmd(nc, in_maps, list(range(N_CORES)))
        except Exception as ex:  # transient device wedges on first exec
            last = ex
    raise last


def kernel(x, e, W0, W1, W2, W3):
    nc = _get_module()
    x = np.ascontiguousarray(x, dtype=np.float32)
    in_maps = [
        {
            "x": x[c * B2 : (c + 1) * B2],
            "e": np.ascontiguousarray(e, dtype=np.float32),
            "W0": np.ascontiguousarray(W0, dtype=np.float32),
            "W1": np.ascontiguousarray(W1, dtype=np.float32),
            "W2": np.ascontiguousarray(W2, dtype=np.float32),
            "W3": np.ascontiguousarray(W3, dtype=np.float32),
        }
        for c in range(N_CORES)
    ]
    res = _run(nc, in_maps)
    out = np.concatenate([res.results[c]["out"] for c in range(N_CORES)], axis=0)
    return out


# revision 13
# speedup vs baseline: 1.2549x; 1.2549x over previous
"""Trainium2 Bass kernel for nn_Causal_Temporal_Map_Attention_2.

Reference computation (B=16, N=2048, T=512, E=64):
    W0m  = relu(triu(W0, 1))
    attn = (x@W0m.T)@x.T + (x@W1.T)@e.T + (e@W2.T)@x.T + (e@W3.T)@e.T
    out  = attn @ x

Associativity avoids the [B, N, N] attention map:
    G[b] = x[b].T @ x[b]                  # [512, 512]  (symmetric!)
    H[b] = e.T @ x[b]                     # [64, 512]
    M0[b] = W0m.T @ G[b] + W1.T @ H[b]    # [512, 512]
    M1[b] = W2.T @ G[b] + W3.T @ H[b]     # [64, 512]
    out[b] = x[b] @ M0[b] + e @ M1[b]     # [2048, 512]

Stage B exploits G's symmetry: only the upper-triangular blocks are
computed, and [x | e] is stored adjacently so the H^T columns ride
along in the same matmuls; the 5 mirror blocks are filled by cheap PE
transposes.  Rows streamed per k-chunk: 1664 vs 2560 naive.

Everything runs in bf16 on the PE (fp32 PSUM accumulation): same or
better streaming rate than f32r, no small-free-dim penalty, 2x faster
weight loads (FWL).  x / x^T / e / e^T are pre-cast and pre-transposed
to bf16 on the host, so the device does no dtype conversions and no
x transposes at all.  Max rel err ~4e-3 (gate is 2e-2).

Sharding: data-parallel over batch, 2 batches per core on 8 cores.
"""

import numpy as np
import ml_dtypes

import concourse.bass as bass
import concourse.mybir as mybir
import concourse.tile as tile
from concourse import bacc
from concourse.bass import ts
from concourse.bass_utils import run_bass_kernel_spmd
from concourse.masks import make_identity

N_CORES = 8
B = 16
B2 = B // N_CORES  # batches per core
N = 2048
T = 512
E = 64
NCHUNKS = N // 128  # 16
KT = T // 128  # 4

f32 = mybir.dt.float32
bf16 = mybir.dt.bfloat16
AF = mybir.ActivationFunctionType
NP_BF16 = ml_dtypes.bfloat16


def build_module():
    nc = bacc.Bacc("TRN2", target_bir_lowering=False, debug=False, num_devices=N_CORES)

    XB = nc.dram_tensor("xb", [B2, N, T], bf16, kind="ExternalInput").ap()
    XT = nc.dram_tensor("xt", [B2, T, N], bf16, kind="ExternalInput").ap()
    EB = nc.dram_tensor("eb", [N, E], bf16, kind="ExternalInput").ap()
    ET = nc.dram_tensor("et", [E, N], bf16, kind="ExternalInput").ap()
    W0 = nc.dram_tensor("W0", [T, T], f32, kind="ExternalInput").ap()
    W1 = nc.dram_tensor("W1", [E, T], f32, kind="ExternalInput").ap()
    W2 = nc.dram_tensor("W2", [T, E], f32, kind="ExternalInput").ap()
    W3 = nc.dram_tensor("W3", [E, E], f32, kind="ExternalInput").ap()
    OUT = nc.dram_tensor("out", [B2, N, T], f32, kind="ExternalOutput").ap()

    TE = T + E  # 576: x chunk with e columns appended

    with tile.TileContext(nc) as tc:
        with (
            tc.tile_pool(name="const", bufs=1) as cpool,
            tc.tile_pool(name="gh", bufs=1) as ghpool,
            tc.tile_pool(name="m01", bufs=1) as mpool,
            tc.tile_pool(name="outst", bufs=4) as opool,
            # PSUM: 5 banks for stage-B accumulators, 2 rotating banks
            # shared by stage C / stage D outputs, 1 for PE transposes.
            tc.tile_pool(name="psacc", bufs=1, space="PSUM") as psacc,
            tc.tile_pool(name="pscd", bufs=2, space="PSUM") as pscd,
            tc.tile_pool(name="pst", bufs=1, space="PSUM") as pst,
        ):
            def _scalar_copy(out, in_):
                nc.scalar.activation(out, in_, AF.Copy)

            def _vector_copy(out, in_):
                nc.vector.tensor_copy(out, in_)

            copy_ps = [_vector_copy, _scalar_copy]  # PSUM-capable engines

            # ---- persistent bf16 staging, one set per batch ----
            # xe: [x | e] adjacently for the fused stage-B matmuls
            xe = [
                [
                    cpool.tile([128, TE], bf16, tag=f"xe{s}_{k}", name=f"xe{s}_{k}")
                    for k in range(NCHUNKS)
                ]
                for s in range(B2)
            ]
            # x^T bands: [t-block, n] per kt
            xtb = [
                [
                    cpool.tile([128, N], bf16, tag=f"xt{s}_{kt}", name=f"xt{s}_{kt}")
                    for kt in range(KT)
                ]
                for s in range(B2)
            ]
            etr = cpool.tile([E, N], bf16)
            nc.sync.dma_start(etr[:], ET[:])
            for k in range(NCHUNKS):
                nc.sync.dma_start(xe[0][k][:, T:TE], EB[ts(k, 128), :])
                nc.sync.dma_start(xe[1][k][:, T:TE], EB[ts(k, 128), :])

            # batch-0 x chunks first: stage B is the first PE work.
            # W0 rides after a few chunks so W0m is ready when C needs it.
            for k in range(6):
                nc.sync.dma_start(xe[0][k][:, 0:T], XB[0, ts(k, 128), :])
            w0_st = cpool.tile([128, KT, T], f32)
            for kt in range(KT):
                nc.sync.dma_start(w0_st[:, kt, :], W0[ts(kt, 128), :])
            for k in range(6, NCHUNKS):
                nc.sync.dma_start(xe[0][k][:, 0:T], XB[0, ts(k, 128), :])
            for kt in range(KT):
                nc.sync.dma_start(xtb[0][kt][:], XT[0, ts(kt, 128), :])

            w1_st = cpool.tile([E, T], f32)
            nc.sync.dma_start(w1_st[:], W1[:])
            w2_st = cpool.tile([128, KT, E], f32)
            for kt in range(KT):
                nc.sync.dma_start(w2_st[:, kt, :], W2[ts(kt, 128), :])
            w3_st = cpool.tile([E, E], f32)
            nc.sync.dma_start(w3_st[:], W3[:])

            # batch-1 x after batch-0 + weights on the DMA queue
            for k in range(NCHUNKS):
                nc.sync.dma_start(xe[1][k][:, 0:T], XB[1, ts(k, 128), :])
            for kt in range(KT):
                nc.sync.dma_start(xtb[1][kt][:], XT[1, ts(kt, 128), :])

            ident32 = cpool.tile([128, 128], f32)
            make_identity(nc, ident32[:])
            identb = cpool.tile([128, 128], bf16)
            nc.vector.tensor_copy(identb[:], ident32[:])

            # ---- weights -> bf16 (W0m = relu(triu(W0,1))) ----
            w0m = cpool.tile([128, KT, T], bf16)
            for kt in range(KT):
                # keep W0[d, t] iff t >= d+1  (d = p + 128*kt)
                nc.gpsimd.affine_select(
                    out=w0_st[:, kt, :],
                    in_=w0_st[:, kt, :],
                    compare_op=mybir.AluOpType.is_ge,
                    fill=0.0,
                    base=-(128 * kt + 1),
                    pattern=[[1, T]],
                    channel_multiplier=-1,
                )
                nc.scalar.activation(w0m[:, kt, :], w0_st[:, kt, :], AF.Relu)
            w1 = cpool.tile([E, T], bf16)
            nc.vector.tensor_copy(w1[:], w1_st[:])
            w2 = cpool.tile([128, KT, E], bf16)
            nc.scalar.activation(w2[:], w2_st[:], AF.Copy)
            w3 = cpool.tile([E, E], bf16)
            nc.vector.tensor_copy(w3[:], w3_st[:])

            def xr(s, k, c0, c1):
                return xe[s][k][:, c0:c1]

            # ---------------- per batch ----------------
            for b in range(B2):
                # Stage B: symmetric G = x^T x upper blocks + H^T columns.
                # Per k-chunk, 5 matmuls (lhsT = x block i, rhs spans
                # [x cols | e cols] so H^T rides along):
                #   mA: i=0, cols [0,256)        -> G(0,0),G(0,1)
                #   mB: i=0, cols [256,576)      -> G(0,2),G(0,3),HT0
                #   mC: i=1, cols [128,576)      -> G(1,1..3),HT1
                #   mD: i=2, cols [256,576)      -> G(2,2),G(2,3),HT2
                #   mE: i=3, cols [256,576)      -> G(3,2),G(3,3),HT3
                pa = psacc.tile([128, 256], f32, tag="pA", name="pa")
                pb = psacc.tile([128, 320], f32, tag="pB", name="pb")
                pc = psacc.tile([128, 448], f32, tag="pC", name="pc")
                pd = psacc.tile([128, 320], f32, tag="pD", name="pd")
                pe = psacc.tile([128, 320], f32, tag="pE", name="pe")
                for k in range(NCHUNKS):
                    st = dict(start=(k == 0), stop=(k == NCHUNKS - 1))
                    nc.tensor.matmul(pa[:], xr(b, k, 0, 128), xr(b, k, 0, 256), **st)
                    nc.tensor.matmul(pb[:], xr(b, k, 0, 128), xr(b, k, 256, TE), **st)
                    nc.tensor.matmul(pc[:], xr(b, k, 128, 256), xr(b, k, 128, TE), **st)
                    nc.tensor.matmul(pd[:], xr(b, k, 256, 384), xr(b, k, 256, TE), **st)
                    nc.tensor.matmul(pe[:], xr(b, k, 384, 512), xr(b, k, 256, TE), **st)

                gh = [
                    ghpool.tile([128, T], bf16, tag=f"g{i}", name=f"g{i}")
                    for i in range(KT)
                ]
                hh = ghpool.tile([E, T], bf16, tag="h", name="h")
                # direct blocks (psum f32 -> sbuf bf16 cast copies)
                _vector_copy(gh[0][:, 0:256], pa[:])
                _scalar_copy(gh[0][:, 256:512], pb[:, 0:256])
                _vector_copy(gh[1][:, 128:512], pc[:, 0:384])
                _scalar_copy(gh[2][:, 256:512], pd[:, 0:256])
                _vector_copy(gh[3][:, 256:512], pe[:, 0:256])
                # mirror blocks via PE transpose (SBUF -> psum -> copy)
                mirrors = [
                    (0, 128, 1, 0),  # G(0,1)^T -> gh1 cols 0:128
                    (0, 256, 2, 0),  # G(0,2)^T -> gh2 cols 0:128
                    (1, 256, 2, 128),  # G(1,2)^T -> gh2 cols 128:256
                    (0, 384, 3, 0),  # G(0,3)^T -> gh3 cols 0:128
                ]
                ptm = pst.tile([128, 640], bf16, tag="pt", name="ptm")
                for mi, (src, scol, dst, dcol) in enumerate(mirrors):
                    nc.tensor.transpose(
                        ptm[:, ts(mi, 128)], gh[src][:, scol : scol + 128], identb[:]
                    )
                for mj, (s2, sc2, d2, dc2) in enumerate(mirrors):
                    copy_ps[mj % 2](gh[d2][:, dc2 : dc2 + 128], ptm[:, ts(mj, 128)])
                # last mirror G(1,3)^T + the 4 H^T blocks share one more
                # pass through the transpose bank
                htmp = ghpool.tile([128, KT, E], bf16, tag="htmp", name="htmp")
                hsrc = [(pb, 256), (pc, 384), (pd, 256), (pe, 256)]
                for i, (src_t, c0) in enumerate(hsrc):
                    copy_ps[i % 2](htmp[:, i, :], src_t[:, c0 : c0 + E])
                ptm2 = pst.tile([128, 640], bf16, tag="pt", name="ptm2")
                nc.tensor.transpose(
                    ptm2[:, 0:128], gh[1][:, 384:512], identb[:]
                )
                for i in range(KT):
                    nc.tensor.transpose(
                        ptm2[:E, 128 + i * 128 : 256 + i * 128],
                        htmp[:, i, :],
                        identb[:],
                    )
                _vector_copy(gh[3][:, 128:256], ptm2[:, 0:128])
                _scalar_copy(hh[:], ptm2[:E, 128:640])

                # Stage C: M0 = W0m^T G + W1^T H ; M1 = W2^T G + W3^T H
                m0 = [
                    mpool.tile([128, T], bf16, tag=f"m0{mc}", name=f"m0{mc}")
                    for mc in range(KT)
                ]
                m1 = mpool.tile([E, T], bf16, tag="m1", name="m1")
                for mc in range(KT):
                    pm = pscd.tile([128, T], f32, tag="cd", name=f"pm{mc}")
                    # W0m strictly upper triangular: block (kt, mc) is
                    # zero unless kt <= mc.
                    for kt in range(mc + 1):
                        nc.tensor.matmul(
                            pm[:],
                            w0m[:, kt, ts(mc, 128)],
                            gh[kt][:],
                            start=(kt == 0),
                            stop=False,
                        )
                    nc.tensor.matmul(
                        pm[:], w1[:, ts(mc, 128)], hh[:], start=False, stop=True
                    )
                    copy_ps[mc % 2](m0[mc][:], pm[:])
                pm1 = pscd.tile([128, T], f32, tag="cd", name="pm1")
                for kt in range(KT):
                    nc.tensor.matmul(
                        pm1[:E, :], w2[:, kt, :], gh[kt][:], start=(kt == 0), stop=False
                    )
                nc.tensor.matmul(pm1[:E, :], w3[:], hh[:], start=False, stop=True)
                _scalar_copy(m1[:], pm1[:E, :])

                # Stage D: out = x @ M0 + e @ M1, chunk by chunk over n.
                for i in range(NCHUNKS):
                    po = pscd.tile([128, T], f32, tag="cd", name="po")
                    nc.tensor.matmul(
                        po[:], etr[:, ts(i, 128)], m1[:], start=True, stop=False
                    )
                    for kt in range(KT):
                        nc.tensor.matmul(
                            po[:],
                            xtb[b][kt][:, ts(i, 128)],
                            m0[kt][:],
                            start=False,
                            stop=(kt == KT - 1),
                        )
                    ot = opool.tile([128, T], f32, tag="ot", name="ot")
                    copy_ps[i % 2](ot[:], po[:])
                    nc.sync.dma_start(OUT[b, ts(i, 128), :], ot[:])

    nc.compile()
    return nc


_CACHE = {}


def _get_module():
    if "nc" not in _CACHE:
        _CACHE["nc"] = build_module()
    return _CACHE["nc"]


def _run(nc, in_maps, tries=3):
    last = None
    for _ in range(tries):
        try:
            return run_bass_kernel_spmd(nc, in_maps, list(range(N_CORES)))
        except Exception as ex:  # transient device wedges on first exec
            last = ex
    raise last


def _in_maps(x, e, W0, W1, W2, W3):
    x = np.ascontiguousarray(x, dtype=np.float32)
    xb = x.astype(NP_BF16)
    xt = np.ascontiguousarray(xb.transpose(0, 2, 1))
    eb = np.asarray(e, dtype=np.float32).astype(NP_BF16)
    et = np.ascontiguousarray(eb.T)
    return [
        {
            "xb": xb[c * B2 : (c + 1) * B2],
            "xt": xt[c * B2 : (c + 1) * B2],
            "eb": eb,
            "et": et,
            "W0": np.ascontiguousarray(W0, dtype=np.float32),
            "W1": np.ascontiguousarray(W1, dtype=np.float32),
            "W2": np.ascontiguousarray(W2, dtype=np.float32),
            "W3": np.ascontiguousarray(W3, dtype=np.float32),
        }
        for c in range(N_CORES)
    ]


def kernel(x, e, W0, W1, W2, W3):
    nc = _get_module()
    in_maps = _in_maps(x, e, W0, W1, W2, W3)
    res = _run(nc, in_maps)
    out = np.concatenate([res.results[c]["out"] for c in range(N_CORES)], axis=0)
    return out
